# revision 30
# baseline (speedup 1.0000x reference)
"""Trainium2 Bass kernel for nn_Attention_20315195310831.

Fused attention block: q/k/v projections, per-head RMS-norm on q/k, masked
softmax with per-head gating, value residual, output projection.

Sharding over 8 NeuronCores: core = 4*b + grp handles batch b and heads
[4*grp, 4*grp+4). Each core computes its partial (attn_out + vx) @ Wo_slice;
the host sums the 4 partials per batch.

Design (cost model: matmul cost = out_free x cycles_per_row; fp8e4
DoubleRow = 0.5 cyc/row and contracts 256/instruction):
- q/k/v projections and scores S^T[k,q] run bf16 (precision-critical:
  per-element quantization error in mean-zero dot products does not average
  down, so fp8 projections cost ~3.5e-2 output error).
- exp(S) is written as fp8e4 (with a bias shift so it can't overflow) and
  AV + softmax denominators run fp8 DoubleRow over aligned kb pairs; the
  denominator is a DoubleRow ones-matmul so both heads share one joint
  [128,512] AV PSUM bank. hl=1 AV runs plain fp8 (DoubleRow can't address
  PSUM partition base 64). Output projection runs bf16.
- Block masks are applied by accumulating {0,-1e30} patterns into the score
  PSUM via identity matmuls (PE). Each (entry, hl) score bank is a single
  accumulation group: one start=True matmul + mask adds (a second start in
  the same 2KB PSUM zero region re-pends the whole bank on hardware).
- exp() is the only table-loaded activation in phase 2; rmsnorm's Sqrt
  lives in phase 1 only (Copy/Square are in every ACT table).
- Epilogue: DVE recip of den, GPSIMD partition_broadcast (dst base 0 only -
  base 64 returns garbage on HW), per-half DVE mult, Pool residual add.
- V^T blocks for AV are built with DMA transpose (bf16) + GPSIMD gated fp8
  casts; AV emission is deferred one pair behind the S matmuls so the
  in-order PE queue never stalls on exp.
"""

import sys

sys.path.insert(0, "/opt/trn_rl_repo")

import numpy as np

B, T, C = 2, 2048, 1024
H, D = 16, 64
EPS = 1e-5
SCALE = 1.0 / 8.0  # 1/sqrt(D)
NCORES = 8
HPC = 4  # heads per core
NG = 2  # head-pair groups per core
CB = C // 128  # 128-contraction chunks
CM = CB // 2  # chunk pairs (DoubleRow)
QT = 4  # q tiles of 512
QW = 512
TBLK = T // 128  # 128-blocks along T
NEG_BIG = -1.0e30

_CACHE = {}


def _analyze_mask(mask01):
    """mask01: bool [T, T], mask01[q, k] True = attend.

    Returns (plan, patterns):
      plan[j] = list of (kb, ql, qh, subs) in ascending kb for q-tile j, where
        [ql, qh) is the local (within 512) column range to compute and
        subs = [(qb_local, pat_idx)] lists 128-wide subblocks needing a mask
        pattern.
      patterns: float32 [npat, 128, 128] additive {0, NEG_BIG} masks in
        [k, q] orientation (accumulated into the score PSUM pre-exp).
    """
    pat_index = {}
    patterns = []

    def pat_id(block_qk):
        # block_qk: bool [128 q, 128 k] -> additive pattern [128 k, 128 q]
        add = np.where(block_qk.T, 0.0, NEG_BIG).astype(np.float32)
        key = add.tobytes()
        if key not in pat_index:
            pat_index[key] = len(patterns)
            patterns.append(add)
        return pat_index[key]

    plan = []
    for j in range(QT):
        entries = []
        for kb in range(TBLK):
            qbs = []
            for qb in range(4):
                blk = mask01[
                    (4 * j + qb) * 128 : (4 * j + qb + 1) * 128,
                    kb * 128 : (kb + 1) * 128,
                ]
                qbs.append(blk)
            anyb = [b.any() for b in qbs]
            if not any(anyb):
                continue
            lo = anyb.index(True)
            hi = 4 - anyb[::-1].index(True)
            entries.append([kb, lo, hi, qbs])
        if entries:
            # widen first entry to the union range so the first
            # PSUM-accumulation matmul covers every column later matmuls
            # (AV and denominator) will touch
            ulo = min(e[1] for e in entries)
            uhi = max(e[2] for e in entries)
            entries[0][1] = ulo
            entries[0][2] = uhi
        final = []
        for kb, lo, hi, qbs in entries:
            subs = []
            for qb in range(lo, hi):
                if not qbs[qb].all():
                    subs.append((qb, pat_id(qbs[qb])))
            final.append((kb, lo * 128, hi * 128, subs))
        plan.append(final)

    if not patterns:
        patterns.append(np.zeros((128, 128), np.float32))
    return plan, np.stack(patterns)


def _build_program(plan, npat, neg_bias):
    import concourse.mybir as mybir
    import concourse.tile as tile
    from concourse import bacc

    f32 = mybir.dt.float32
    f32r = mybir.dt.float32r
    bf16 = mybir.dt.bfloat16
    fp8 = mybir.dt.float8e4
    AF = mybir.ActivationFunctionType
    OP = mybir.AluOpType
    DR = mybir.MatmulPerfMode.DoubleRow

    nc = bacc.Bacc(
        "TRN2",
        target_bir_lowering=False,
        debug=False,
        enable_asserts=False,
        num_devices=NCORES,
    )

    x16_d = nc.dram_tensor("x16", [128, CB * T], bf16, kind="ExternalInput").ap()
    wq_d = nc.dram_tensor("wq16", [128, CB * 256], bf16, kind="ExternalInput").ap()
    wk_d = nc.dram_tensor("wk16", [128, CB * 256], bf16, kind="ExternalInput").ap()
    wv_d = nc.dram_tensor("wv16", [128, CB * 256], bf16, kind="ExternalInput").ap()
    wo_d = nc.dram_tensor("wo8", [128, 2 * 1024], bf16, kind="ExternalInput").ap()
    wqc_d = nc.dram_tensor("wq_col", [128, 1], f32, kind="ExternalInput").ap()
    wkc_d = nc.dram_tensor("wk_col", [128, 1], f32, kind="ExternalInput").ap()
    sel2_d = nc.dram_tensor("sel2b", [128, 2], bf16, kind="ExternalInput").ap()
    selT_d = nc.dram_tensor("selTr", [2, 128], f32r, kind="ExternalInput").ap()
    gc4_d = nc.dram_tensor("gcol4", [128, 4], f32, kind="ExternalInput").ap()
    ident_d = nc.dram_tensor("ident16", [128, 128], bf16, kind="ExternalInput").ap()
    pats_d = nc.dram_tensor("patB", [128, 128 * npat], bf16, kind="ExternalInput").ap()
    out_d = nc.dram_tensor("out", [T, C], f32, kind="ExternalOutput").ap()

    with tile.TileContext(nc) as tc:
        with tc.tile_pool(name="pers", bufs=1) as pers:
            qT = [pers.tile([128, T], bf16, tag=f"qT{g}", name=f"qT{g}") for g in range(NG)]
            kT = [pers.tile([128, T], bf16, tag=f"kT{g}", name=f"kT{g}") for g in range(NG)]
            vT = [pers.tile([128, T], bf16, tag=f"vT{g}", name=f"vT{g}") for g in range(NG)]
            # vaug8[m]: [128 kpos, pair i, head h, 64 dims] fp8, kb = 2m+i
            vaug = [
                pers.tile([128, 2, HPC, 64], fp8, tag=f"va{m}", name=f"va{m}")
                for m in range(TBLK // 2)
            ]
            AT8 = pers.tile([128, NG, T], bf16, tag="AT8", name="AT8")
            wo_sb = pers.tile([128, 2, 1024], bf16, tag="wo_sb", name="wo_sb")
            wq_col = pers.tile([128, 1], f32, tag="wqc", name="wqc")
            wk_col = pers.tile([128, 1], f32, tag="wkc", name="wkc")
            sel2 = pers.tile([128, 2], bf16, tag="sel2", name="sel2")
            selT = pers.tile([2, 128], f32r, tag="selT", name="selT")
            gcol4 = pers.tile([128, 4], f32, tag="gc4", name="gc4")
            ident = pers.tile([128, 128], bf16, tag="ident", name="ident")
            pats = pers.tile([128, 128 * npat], bf16, tag="pats", name="pats")
            eps_col = pers.tile([2, 1], f32, tag="eps", name="eps")
            nb_col = pers.tile([128, 1], f32, tag="nb", name="nb")
            ones8 = pers.tile([128, 2, 16], fp8, tag="ones8", name="ones8")
            nc.vector.memset(eps_col, EPS)
            nc.vector.memset(nb_col, neg_bias)
            nc.vector.memset(ones8, 1.0)

            # ---- phase 1: projections + q/k rmsnorm + V transposes
            with tc.tile_pool(name="xw", bufs=1) as xw, \
                 tc.tile_pool(name="p1sb", bufs=4) as p1sb, \
                 tc.tile_pool(name="p1ps", bufs=4, space="PSUM") as p1ps, \
                 tc.tile_pool(name="p1pss", bufs=2, space="PSUM") as p1pss:
                w_sb = {}
                for nm, d in (("wv", wv_d), ("wq", wq_d), ("wk", wk_d)):
                    w16 = xw.tile([128, CB, 256], bf16, tag=f"{nm}s", name=f"{nm}s")
                    nc.sync.dma_start(w16.rearrange("p a b -> p (a b)"), d)
                    w_sb[nm] = w16
                x16t = xw.tile([128, CB, T], bf16, tag="x16t", name="x16t")
                x16v = x16_d.rearrange("p (c t) -> p c t", c=CB)
                for nk in range(QT):
                    cs0 = slice(QW * nk, QW * (nk + 1))
                    for c in range(CB):
                        nc.sync.dma_start(x16t[:, c, cs0], x16v[:, c, cs0])
                    if nk == 0:
                        nc.sync.dma_start(sel2, sel2_d)
                        nc.sync.dma_start(selT, selT_d)
                        nc.sync.dma_start(wq_col, wqc_d)
                        nc.sync.dma_start(wk_col, wkc_d)
                    elif nk == 1:
                        nc.sync.dma_start(gcol4, gc4_d)
                        nc.sync.dma_start(ident, ident_d)
                        nc.sync.dma_start(pats, pats_d)
                    elif nk == 2:
                        nc.sync.dma_start(
                            wo_sb.rearrange("p a b -> p (a b)"), wo_d
                        )

                for nk in range(QT):
                    cs = slice(QW * nk, QW * (nk + 1))
                    for nm, isqk, outT, wcol in (
                        ("wv", False, vT, None),
                        ("wq", True, qT, wq_col),
                        ("wk", True, kT, wk_col),
                    ):
                        for g in range(NG):
                            pp = p1ps.tile([128, QW], f32, tag="proj", name="pp")
                            for c in range(CB):
                                nc.tensor.matmul(
                                    pp,
                                    w_sb[nm][:, c, 128 * g : 128 * (g + 1)],
                                    x16t[:, c, cs],
                                    start=(c == 0),
                                    stop=(c == CB - 1),
                                )
                            if not isqk:
                                # V: copy to bf16 vT (ACT, table-free Copy)
                                nc.scalar.copy(vT[g][:, cs], pp)
                                continue
                            # q/k rmsnorm: pp -> bf16 SBUF copy (ACT), square
                            # on DVE, mean via sel2 matmul (entries 1/64),
                            # sqrt on ACT, recip on DVE, rms broadcast via
                            # selT matmul, single-PSUM-input apply on DVE
                            ppb = p1sb.tile([128, QW], bf16, tag="ppb", name="ppb", bufs=6)
                            nc.scalar.copy(ppb, pp)
                            sq = p1sb.tile([128, QW], bf16, tag="sq", name="sq", bufs=6)
                            nc.vector.tensor_tensor(sq, ppb, ppb, OP.mult)
                            st = p1pss.tile([2, QW], f32, tag="st", name="st", bufs=2)
                            nc.tensor.matmul(st, sel2, sq, start=True, stop=True)
                            msq = p1sb.tile([2, QW], f32r, tag="ms", name="ms", bufs=4)
                            nc.scalar.activation(msq, st, AF.Sqrt, bias=eps_col)
                            rms = p1sb.tile([2, QW], f32r, tag="rm", name="rm", bufs=4)
                            with nc.allow_low_precision(reason="f32r recip feeds matmul"):
                                nc.vector.reciprocal(rms, msq)
                            mm = p1pss.tile([128, QW], f32, tag="mm", name="mm", bufs=2)
                            nc.tensor.matmul(mm, selT, rms, start=True, stop=True)
                            nc.vector.scalar_tensor_tensor(
                                outT[g][:, cs], ppb, wcol, mm, OP.mult, OP.mult
                            )
                    # V transposes for this chunk: kb = 4*nk .. 4*nk+3.
                    # The per-head gate is folded into vaug here (numerator
                    # only; den uses the separate ones8 lhsT and vT carries
                    # the ungated residual).
                    for g in range(NG):
                        for kb in range(4 * nk, 4 * nk + 4):
                            stg = p1sb.tile([128, 128], bf16, tag="stg", name="stg", bufs=4)
                            nc.sync.dma_start(
                                stg, vT[g][:, 128 * kb : 128 * (kb + 1)],
                                transpose=True,
                            )
                            for hl in range(2):
                                h = 2 * g + hl
                                nc.gpsimd.tensor_scalar(
                                    vaug[kb // 2][:, kb % 2, h, :],
                                    stg[:, 64 * hl : 64 * (hl + 1)],
                                    gcol4[:, h : h + 1],
                                    None,
                                    OP.mult,
                                )

            # ---- phase 2: attention + deferred output projection
            with tc.tile_pool(name="p2sb", bufs=1) as p2sb, \
                 tc.tile_pool(name="p4sb", bufs=1) as p4sb, \
                 tc.tile_pool(name="psS", bufs=2, space="PSUM") as psS, \
                 tc.tile_pool(name="psO", bufs=1, space="PSUM") as psO, \
                 tc.tile_pool(name="psA", bufs=2, space="PSUM") as psA, \
                 tc.tile_pool(name="psP", bufs=1, space="PSUM") as psP:

                pending_oproj = []

                def emit_oproj(final=False):
                    cnt = 0
                    while pending_oproj:
                        tb = pending_oproj.pop(0)
                        for nn in range(2):
                            if final and cnt % 2 == 1:
                                # tail flush: borrow a dead s_ps slot so two
                                # po banks pipeline, and put half the copies
                                # on ACT (idle after the last exp; Copy is
                                # in every activation table)
                                pw = psS.tile(
                                    [128, 2, QW], f32, tag="s", name="po_b"
                                )
                                po = pw[:, 0, :]
                            else:
                                po = psP.tile([128, QW], f32, tag="po", name="po")
                            for cg in range(2):
                                nc.tensor.matmul(
                                    po,
                                    AT8[:, cg, 128 * tb : 128 * (tb + 1)],
                                    wo_sb[:, cg, QW * nn : QW * (nn + 1)],
                                    start=cg == 0,
                                    stop=cg == 1,
                                )
                            ob = p4sb.tile([128, QW], f32, tag="ob", name="ob", bufs=3)
                            if final and cnt % 2 == 1:
                                nc.scalar.copy(ob, po)
                            else:
                                nc.vector.tensor_copy(ob, po)
                            cnt += 1
                            nc.sync.dma_start(
                                out_d[128 * tb : 128 * (tb + 1), QW * nn : QW * (nn + 1)],
                                ob,
                            )

                for j in range(QT):
                    entries = plan[j]
                    if not entries:
                        continue
                    # group entries into aligned DoubleRow pairs (kb=2m, 2m+1)
                    pairs = []
                    i = 0
                    while i < len(entries):
                        e = entries[i]
                        if (
                            i + 1 < len(entries)
                            and e[0] % 2 == 0
                            and entries[i + 1][0] == e[0] + 1
                        ):
                            pairs.append((e, entries[i + 1]))
                            i += 2
                        else:
                            pairs.append((e, None))
                            i += 1
                    qs = slice(QW * j, QW * (j + 1))
                    for g in range(NG):
                        o_ps = psO.tile([128, QW], f32, tag="o", name="o_ps")
                        den_ps = [
                            psA.tile([1, QW], f32, tag=f"den{hl}", name=f"den{hl}", bufs=1)
                            for hl in range(2)
                        ]
                        npair = len(pairs)
                        def emit_av(pt8, ea, eb, qlp, qhp, first, last):
                            if eb is not None:
                                # hl=0 and both denominators: DoubleRow over
                                # the aligned kb pair. hl=1 writes PSUM
                                # partitions 64.. which DoubleRow can't
                                # address (no col tile_position), so it runs
                                # plain-fp8 per entry.
                                m = ea[0] // 2
                                nc.tensor.matmul(
                                    o_ps[0:64, qlp:qhp],
                                    vaug[m][:, :, 2 * g, :],
                                    pt8[:, :, 0, qlp:qhp],
                                    start=first,
                                    stop=last,
                                    perf_mode=DR,
                                    skip_group_check=True,
                                )
                                for hl in range(2):
                                    nc.tensor.matmul(
                                        den_ps[hl][:, qlp:qhp],
                                        ones8[:, :, 0:1],
                                        pt8[:, :, hl, qlp:qhp],
                                        start=first,
                                        stop=last,
                                        perf_mode=DR,
                                        skip_group_check=True,
                                    )
                                for ei2, e2 in enumerate((ea, eb)):
                                    kb2 = e2[0]
                                    nc.tensor.matmul(
                                        o_ps[64:128, qlp:qhp],
                                        vaug[kb2 // 2][:, kb2 % 2, 2 * g + 1, :],
                                        pt8[:, ei2, 1, qlp:qhp],
                                        start=first and ei2 == 0,
                                        stop=last and ei2 == 1,
                                        skip_group_check=True,
                                    )
                            else:
                                kb = ea[0]
                                for hl in range(2):
                                    nc.tensor.matmul(
                                        o_ps[64 * hl : 64 * (hl + 1), qlp:qhp],
                                        vaug[kb // 2][:, kb % 2, 2 * g + hl, :],
                                        pt8[:, 0, hl, qlp:qhp],
                                        start=first,
                                        stop=last,
                                        skip_group_check=True,
                                    )
                                    nc.tensor.matmul(
                                        den_ps[hl][:, qlp:qhp],
                                        ones8[:, kb % 2, 0:1],
                                        pt8[:, 0, hl, qlp:qhp],
                                        start=first,
                                        stop=last,
                                        skip_group_check=True,
                                    )

                        pend_av = None
                        for pi, (ea, eb) in enumerate(pairs):
                            pt8 = p2sb.tile(
                                [128, 2, 2, QW], fp8, tag="pt8", name="pt8", bufs=6
                            )
                            qlp = min(e[1] for e in (ea, eb) if e)
                            qhp = max(e[2] for e in (ea, eb) if e)
                            for ei, e in enumerate((ea, eb)):
                                if e is None:
                                    continue
                                kb, ql, qh, subs = e
                                s_ps = psS.tile(
                                    [128, 2, QW], f32, tag="s", name="s_ps"
                                )
                                # one accumulation group per (entry, hl) bank:
                                # a single start=True S matmul over [ql,qh),
                                # then mask patterns accumulate; the last
                                # instruction carries stop=True (a second
                                # start in the same PSUM bank would re-pend
                                # the whole 2KB zero region on hardware)
                                for hl in range(2):
                                    rs = slice(64 * hl, 64 * (hl + 1))
                                    nc.tensor.matmul(
                                        s_ps[:, hl, ql:qh],
                                        kT[g][rs, 128 * kb : 128 * (kb + 1)],
                                        qT[g][rs, QW * j + ql : QW * j + qh],
                                        start=True,
                                        stop=not subs,
                                        tile_position=(64 * hl, 0),
                                    )
                                    for si, (qb, pidx) in enumerate(subs):
                                        nc.tensor.matmul(
                                            s_ps[:, hl, 128 * qb : 128 * (qb + 1)],
                                            ident,
                                            pats[:, 128 * pidx : 128 * (pidx + 1)],
                                            start=False,
                                            stop=si == len(subs) - 1,
                                        )
                                # zero pt8 outside [ql,qh) within pair range
                                if ql > qlp:
                                    nc.gpsimd.memset(pt8[:, ei, :, qlp:ql], 0.0)
                                if qh < qhp:
                                    nc.gpsimd.memset(pt8[:, ei, :, qh:qhp], 0.0)
                                nc.scalar.activation(
                                    pt8[:, ei, :, ql:qh],
                                    s_ps[:, :, ql:qh],
                                    AF.Exp,
                                    bias=nb_col,
                                    scale=SCALE,
                                )
                            # defer this pair's AV/den behind the next pair's
                            # S matmuls so the in-order PE queue never stalls
                            # on this pair's exp
                            if pend_av is not None:
                                pend_av()
                            pend_av = (
                                lambda pt8=pt8, ea=ea, eb=eb, qlp=qlp, qhp=qhp,
                                first=pi == 0, last=pi == npair - 1:
                                emit_av(pt8, ea, eb, qlp, qhp, first, last)
                            )
                            if pi == 1:
                                emit_oproj()
                        pend_av()
                        # epilogue: AT = o_ps (gated) / den_bcast + vT.
                        # partition_broadcast only writes correctly at dst
                        # base 0, so each head's recip gets a full 128-row
                        # broadcast and the mult runs per 64-row half.
                        tmp16 = p4sb.tile([128, QW], bf16, tag="tmp", name="tmp", bufs=3)
                        for hl in range(2):
                            rd = p4sb.tile([1, QW], f32, tag="rd", name="rd", bufs=4)
                            nc.vector.reciprocal(rd, den_ps[hl])
                            m2sb = p4sb.tile([128, QW], f32, tag="m2s", name="m2s", bufs=2)
                            nc.gpsimd.partition_broadcast(m2sb, rd, channels=128)
                            rs = slice(64 * hl, 64 * (hl + 1))
                            nc.vector.tensor_tensor(
                                tmp16[rs, :], o_ps[rs, :], m2sb[rs, :], OP.mult
                            )
                        nc.gpsimd.tensor_tensor(
                            AT8[:, g, qs], tmp16, vT[g][:, qs], OP.add
                        )
                    pending_oproj.extend(range(4 * j, 4 * j + 4))
                emit_oproj(final=True)

    nc.compile()
    return nc


def kernel(**inputs):
    import ml_dtypes

    from concourse import bass_utils

    NPF8 = ml_dtypes.float8_e4m3fn
    NPBF = ml_dtypes.bfloat16

    x = np.asarray(inputs["x"], np.float32)
    mask = np.asarray(inputs["attention_mask"])
    Wq = np.asarray(inputs["Wq"], np.float32)
    Wk = np.asarray(inputs["Wk"], np.float32)
    Wv = np.asarray(inputs["Wv"], np.float32)
    Wo = np.asarray(inputs["Wo"], np.float32)
    qw = np.asarray(inputs["q_norm_w"], np.float32)
    kw = np.asarray(inputs["k_norm_w"], np.float32)
    gate = np.asarray(inputs["gate"], np.float32).reshape(H)

    mask01 = mask.reshape(T, T) != 0
    plan, patterns = _analyze_mask(mask01)
    npat = patterns.shape[0]

    # shift exp so fp8e4 output can't overflow (cancels in normalization)
    bound = 8.0 * np.max(np.abs(qw)) * np.max(np.abs(kw))
    neg_bias = -max(0.0, float(bound) - 6.0)

    key = (hash(mask01.tobytes()), npat, neg_bias)
    if key not in _CACHE:
        _CACHE[key] = _build_program(plan, npat, neg_bias)
    nc = _CACHE[key]

    pats_r = np.ascontiguousarray(
        patterns.transpose(1, 0, 2).reshape(128, 128 * npat)
    ).astype(NPBF)
    sel2 = np.zeros((128, 2), np.float32)
    sel2[0:64, 0] = 1.0 / D
    sel2[64:128, 1] = 1.0 / D
    sel2 = sel2.astype(NPBF)
    selT = np.zeros((2, 128), np.float32)
    selT[0, 0:64] = 1.0
    selT[1, 64:128] = 1.0
    ident16 = np.eye(128, dtype=np.float32).astype(NPBF)
    wq_col = np.tile(qw, 2)[:, None].astype(np.float32)
    wk_col = np.tile(kw, 2)[:, None].astype(np.float32)

    def chunk_major16(W):
        # [1024, n] -> [128, CB*n] bf16, 128-row chunks side by side
        ci, n = W.shape
        return np.ascontiguousarray(
            W.reshape(CB, 128, n).transpose(1, 0, 2).reshape(128, -1)
        ).astype(NPBF)

    in_maps = []
    for core in range(NCORES):
        b, grp = core // 4, core % 4
        hs = slice(256 * grp, 256 * (grp + 1))
        gcol4 = np.ascontiguousarray(
            np.broadcast_to(gate[4 * grp : 4 * grp + 4][None, :], (128, HPC))
        ).astype(np.float32)
        wo8 = np.ascontiguousarray(
            Wo[hs, :].reshape(2, 128, 1024).transpose(1, 0, 2).reshape(128, -1)
        ).astype(NPBF)
        in_maps.append(
            {
                "x16": chunk_major16(x[b].T),
                "wq16": chunk_major16(Wq[:, hs]),
                "wk16": chunk_major16(Wk[:, hs]),
                "wv16": chunk_major16(Wv[:, hs]),
                "wo8": wo8,
                "wq_col": wq_col,
                "wk_col": wk_col,
                "sel2b": sel2,
                "selTr": selT,
                "gcol4": gcol4,
                "ident16": ident16,
                "patB": pats_r,
            }
        )

    global _LAST_IN_MAPS
    _LAST_IN_MAPS = in_maps
    res = bass_utils.run_bass_kernel_spmd(nc, in_maps, core_ids=list(range(NCORES)))
    parts = [res.results[i]["out"] for i in range(NCORES)]
    out = np.stack(
        [
            parts[0] + parts[1] + parts[2] + parts[3],
            parts[4] + parts[5] + parts[6] + parts[7],
        ]
    )
    return out.astype(np.float32)


# revision 32
# speedup vs baseline: 1.0360x; 1.0360x over previous
"""Trainium2 Bass kernel for nn_Attention_20315195310831.

Fused attention block: q/k/v projections, per-head RMS-norm on q/k, masked
softmax with per-head gating, value residual, output projection.

Sharding over 8 NeuronCores: core = 4*b + grp handles batch b and heads
[4*grp, 4*grp+4). Each core computes its partial (attn_out + vx) @ Wo_slice;
the host sums the 4 partials per batch.

Design (cost model: matmul cost = out_free x cycles_per_row; fp8e4
DoubleRow = 0.5 cyc/row and contracts 256/instruction):
- q/k/v projections and scores S^T[k,q] run bf16 (precision-critical:
  per-element quantization error in mean-zero dot products does not average
  down, so fp8 projections cost ~3.5e-2 output error).
- exp(S) is written as fp8e4 (with a bias shift so it can't overflow) and
  AV + softmax denominators run fp8 DoubleRow over aligned kb pairs; the
  denominator is a DoubleRow ones-matmul so both heads share one joint
  [128,512] AV PSUM bank. hl=1 AV runs plain fp8 (DoubleRow can't address
  PSUM partition base 64). Output projection runs bf16.
- Block masks are applied by accumulating {0,-1e30} patterns into the score
  PSUM via identity matmuls (PE). Each (entry, hl) score bank is a single
  accumulation group: one start=True matmul + mask adds (a second start in
  the same 2KB PSUM zero region re-pends the whole bank on hardware).
- exp() is the only table-loaded activation in phase 2; rmsnorm's Sqrt
  lives in phase 1 only (Copy/Square are in every ACT table).
- Epilogue: DVE recip of den, GPSIMD partition_broadcast (dst base 0 only -
  base 64 returns garbage on HW), per-half DVE mult, Pool residual add.
- V^T blocks for AV are built with DMA transpose (bf16) + GPSIMD gated fp8
  casts; AV emission is deferred one pair behind the S matmuls so the
  in-order PE queue never stalls on exp.
"""

import sys

sys.path.insert(0, "/opt/trn_rl_repo")

import numpy as np

B, T, C = 2, 2048, 1024
H, D = 16, 64
EPS = 1e-5
SCALE = 1.0 / 8.0  # 1/sqrt(D)
NCORES = 8
HPC = 4  # heads per core
NG = 2  # head-pair groups per core
CB = C // 128  # 128-contraction chunks
CM = CB // 2  # chunk pairs (DoubleRow)
QT = 4  # q tiles of 512
QW = 512
TBLK = T // 128  # 128-blocks along T
NEG_BIG = -1.0e30

_CACHE = {}


def _analyze_mask(mask01):
    """mask01: bool [T, T], mask01[q, k] True = attend.

    Returns (plan, patterns):
      plan[j] = list of (kb, ql, qh, subs) in ascending kb for q-tile j, where
        [ql, qh) is the local (within 512) column range to compute and
        subs = [(qb_local, pat_idx)] lists 128-wide subblocks needing a mask
        pattern.
      patterns: float32 [npat, 128, 128] additive {0, NEG_BIG} masks in
        [k, q] orientation (accumulated into the score PSUM pre-exp).
    """
    pat_index = {}
    patterns = []

    def pat_id(block_qk):
        # block_qk: bool [128 q, 128 k] -> additive pattern [128 k, 128 q]
        add = np.where(block_qk.T, 0.0, NEG_BIG).astype(np.float32)
        key = add.tobytes()
        if key not in pat_index:
            pat_index[key] = len(patterns)
            patterns.append(add)
        return pat_index[key]

    plan = []
    for j in range(QT):
        entries = []
        for kb in range(TBLK):
            qbs = []
            for qb in range(4):
                blk = mask01[
                    (4 * j + qb) * 128 : (4 * j + qb + 1) * 128,
                    kb * 128 : (kb + 1) * 128,
                ]
                qbs.append(blk)
            anyb = [b.any() for b in qbs]
            if not any(anyb):
                continue
            lo = anyb.index(True)
            hi = 4 - anyb[::-1].index(True)
            entries.append([kb, lo, hi, qbs])
        if entries:
            # widen first entry to the union range so the first
            # PSUM-accumulation matmul covers every column later matmuls
            # (AV and denominator) will touch
            ulo = min(e[1] for e in entries)
            uhi = max(e[2] for e in entries)
            entries[0][1] = ulo
            entries[0][2] = uhi
        final = []
        for kb, lo, hi, qbs in entries:
            subs = []
            for qb in range(lo, hi):
                if not qbs[qb].all():
                    subs.append((qb, pat_id(qbs[qb])))
            final.append((kb, lo * 128, hi * 128, subs))
        plan.append(final)

    if not patterns:
        patterns.append(np.zeros((128, 128), np.float32))
    return plan, np.stack(patterns)


def _build_program(plan, npat, neg_bias):
    import concourse.mybir as mybir
    import concourse.tile as tile
    from concourse import bacc

    f32 = mybir.dt.float32
    f32r = mybir.dt.float32r
    bf16 = mybir.dt.bfloat16
    fp8 = mybir.dt.float8e4
    AF = mybir.ActivationFunctionType
    OP = mybir.AluOpType
    DR = mybir.MatmulPerfMode.DoubleRow

    nc = bacc.Bacc(
        "TRN2",
        target_bir_lowering=False,
        debug=False,
        enable_asserts=False,
        num_devices=NCORES,
    )

    x16_d = nc.dram_tensor("x16", [128, CB * T], bf16, kind="ExternalInput").ap()
    wq_d = nc.dram_tensor("wq16", [128, CB * 256], bf16, kind="ExternalInput").ap()
    wk_d = nc.dram_tensor("wk16", [128, CB * 256], bf16, kind="ExternalInput").ap()
    wv_d = nc.dram_tensor("wv16", [128, CB * 256], bf16, kind="ExternalInput").ap()
    wo_d = nc.dram_tensor("wo8", [128, 2 * 1024], bf16, kind="ExternalInput").ap()
    wqc_d = nc.dram_tensor("wq_col", [128, 1], f32, kind="ExternalInput").ap()
    wkc_d = nc.dram_tensor("wk_col", [128, 1], f32, kind="ExternalInput").ap()
    sel2_d = nc.dram_tensor("sel2b", [128, 2], bf16, kind="ExternalInput").ap()
    selT_d = nc.dram_tensor("selTr", [2, 128], f32r, kind="ExternalInput").ap()
    gc4_d = nc.dram_tensor("gcol4", [128, 4], f32, kind="ExternalInput").ap()
    ident_d = nc.dram_tensor("ident16", [128, 128], bf16, kind="ExternalInput").ap()
    pats_d = nc.dram_tensor("patB", [128, 128 * npat], bf16, kind="ExternalInput").ap()
    out_d = nc.dram_tensor("out", [T, C], f32, kind="ExternalOutput").ap()

    with tile.TileContext(nc) as tc:
        with tc.tile_pool(name="pers", bufs=1) as pers:
            qT = [pers.tile([128, T], bf16, tag=f"qT{g}", name=f"qT{g}") for g in range(NG)]
            kT = [pers.tile([128, T], bf16, tag=f"kT{g}", name=f"kT{g}") for g in range(NG)]
            vT = [pers.tile([128, T], bf16, tag=f"vT{g}", name=f"vT{g}") for g in range(NG)]
            # vaug8[m]: [128 kpos, pair i, head h, 64 dims] fp8, kb = 2m+i
            vaug = [
                pers.tile([128, 2, HPC, 64], fp8, tag=f"va{m}", name=f"va{m}")
                for m in range(TBLK // 2)
            ]
            AT8 = pers.tile([128, NG, T], bf16, tag="AT8", name="AT8")
            wo_sb = pers.tile([128, 2, 1024], bf16, tag="wo_sb", name="wo_sb")
            wq_col = pers.tile([128, 1], f32, tag="wqc", name="wqc")
            wk_col = pers.tile([128, 1], f32, tag="wkc", name="wkc")
            sel2 = pers.tile([128, 2], bf16, tag="sel2", name="sel2")
            selT = pers.tile([2, 128], f32r, tag="selT", name="selT")
            gcol4 = pers.tile([128, 4], f32, tag="gc4", name="gc4")
            ident = pers.tile([128, 128], bf16, tag="ident", name="ident")
            pats = pers.tile([128, 128 * npat], bf16, tag="pats", name="pats")
            eps_col = pers.tile([2, 1], f32, tag="eps", name="eps")
            nb_col = pers.tile([128, 1], f32, tag="nb", name="nb")
            ones8 = pers.tile([128, 2, 16], fp8, tag="ones8", name="ones8")
            nc.vector.memset(eps_col, EPS)
            nc.vector.memset(nb_col, neg_bias)
            nc.vector.memset(ones8, 1.0)

            # ---- phase 1: projections + q/k rmsnorm + V transposes
            with tc.tile_pool(name="xw", bufs=1) as xw, \
                 tc.tile_pool(name="p1sb", bufs=4) as p1sb, \
                 tc.tile_pool(name="p1ps", bufs=4, space="PSUM") as p1ps, \
                 tc.tile_pool(name="p1pss", bufs=2, space="PSUM") as p1pss:
                # nk=0 x-chunks go first on the SP queue; the big weight
                # loads stream concurrently on the DVE queue so the first
                # projection chain starts ~2us in instead of ~7us
                x16t = xw.tile([128, CB, T], bf16, tag="x16t", name="x16t")
                x16v = x16_d.rearrange("p (c t) -> p c t", c=CB)
                for c in range(CB):
                    nc.sync.dma_start(x16t[:, c, 0:QW], x16v[:, c, 0:QW])
                nc.sync.dma_start(sel2, sel2_d)
                nc.sync.dma_start(selT, selT_d)
                nc.sync.dma_start(wq_col, wqc_d)
                nc.sync.dma_start(wk_col, wkc_d)
                w_sb = {}
                for nm, d in (("wv", wv_d), ("wq", wq_d), ("wk", wk_d)):
                    w16 = xw.tile([128, CB, 256], bf16, tag=f"{nm}s", name=f"{nm}s")
                    nc.scalar.dma_start(w16.rearrange("p a b -> p (a b)"), d)
                    w_sb[nm] = w16
                for nk in range(1, QT):
                    cs0 = slice(QW * nk, QW * (nk + 1))
                    for c in range(CB):
                        nc.sync.dma_start(x16t[:, c, cs0], x16v[:, c, cs0])
                    if nk == 1:
                        nc.sync.dma_start(gcol4, gc4_d)
                        nc.sync.dma_start(ident, ident_d)
                        nc.sync.dma_start(pats, pats_d)
                    elif nk == 2:
                        nc.sync.dma_start(
                            wo_sb.rearrange("p a b -> p (a b)"), wo_d
                        )

                for nk in range(QT):
                    cs = slice(QW * nk, QW * (nk + 1))
                    for nm, isqk, outT, wcol in (
                        ("wv", False, vT, None),
                        ("wq", True, qT, wq_col),
                        ("wk", True, kT, wk_col),
                    ):
                        for g in range(NG):
                            pp = p1ps.tile([128, QW], f32, tag="proj", name="pp")
                            for c in range(CB):
                                nc.tensor.matmul(
                                    pp,
                                    w_sb[nm][:, c, 128 * g : 128 * (g + 1)],
                                    x16t[:, c, cs],
                                    start=(c == 0),
                                    stop=(c == CB - 1),
                                )
                            if not isqk:
                                # V: copy to bf16 vT (ACT, table-free Copy)
                                nc.scalar.copy(vT[g][:, cs], pp)
                                continue
                            # q/k rmsnorm: pp -> bf16 SBUF copy (ACT), square
                            # on DVE, mean via sel2 matmul (entries 1/64),
                            # sqrt on ACT, recip on DVE, rms broadcast via
                            # selT matmul, single-PSUM-input apply on DVE
                            ppb = p1sb.tile([128, QW], bf16, tag="ppb", name="ppb", bufs=6)
                            nc.scalar.copy(ppb, pp)
                            sq = p1sb.tile([128, QW], bf16, tag="sq", name="sq", bufs=6)
                            nc.vector.tensor_tensor(sq, ppb, ppb, OP.mult)
                            st = p1pss.tile([2, QW], f32, tag="st", name="st", bufs=2)
                            nc.tensor.matmul(st, sel2, sq, start=True, stop=True)
                            msq = p1sb.tile([2, QW], f32r, tag="ms", name="ms", bufs=4)
                            nc.scalar.activation(msq, st, AF.Sqrt, bias=eps_col)
                            rms = p1sb.tile([2, QW], f32r, tag="rm", name="rm", bufs=4)
                            with nc.allow_low_precision(reason="f32r recip feeds matmul"):
                                nc.vector.reciprocal(rms, msq)
                            mm = p1pss.tile([128, QW], f32, tag="mm", name="mm", bufs=2)
                            nc.tensor.matmul(mm, selT, rms, start=True, stop=True)
                            nc.vector.scalar_tensor_tensor(
                                outT[g][:, cs], ppb, wcol, mm, OP.mult, OP.mult
                            )
                    # V transposes for this chunk: kb = 4*nk .. 4*nk+3.
                    # The per-head gate is folded into vaug here (numerator
                    # only; den uses the separate ones8 lhsT and vT carries
                    # the ungated residual).
                    for g in range(NG):
                        for kb in range(4 * nk, 4 * nk + 4):
                            stg = p1sb.tile([128, 128], bf16, tag="stg", name="stg", bufs=4)
                            nc.sync.dma_start(
                                stg, vT[g][:, 128 * kb : 128 * (kb + 1)],
                                transpose=True,
                            )
                            for hl in range(2):
                                h = 2 * g + hl
                                nc.gpsimd.tensor_scalar(
                                    vaug[kb // 2][:, kb % 2, h, :],
                                    stg[:, 64 * hl : 64 * (hl + 1)],
                                    gcol4[:, h : h + 1],
                                    None,
                                    OP.mult,
                                )

            # ---- phase 2: attention + deferred output projection
            with tc.tile_pool(name="p2sb", bufs=1) as p2sb, \
                 tc.tile_pool(name="p4sb", bufs=1) as p4sb, \
                 tc.tile_pool(name="psS", bufs=2, space="PSUM") as psS, \
                 tc.tile_pool(name="psO", bufs=1, space="PSUM") as psO, \
                 tc.tile_pool(name="psA", bufs=2, space="PSUM") as psA, \
                 tc.tile_pool(name="psP", bufs=1, space="PSUM") as psP:

                pending_oproj = []

                def emit_oproj(final=False):
                    cnt = 0
                    while pending_oproj:
                        tb = pending_oproj.pop(0)
                        for nn in range(2):
                            if final and cnt % 2 == 1:
                                # tail flush: borrow a dead s_ps slot so two
                                # po banks pipeline, and put half the copies
                                # on ACT (idle after the last exp; Copy is
                                # in every activation table)
                                pw = psS.tile(
                                    [128, 2, QW], f32, tag="s", name="po_b"
                                )
                                po = pw[:, 0, :]
                            else:
                                po = psP.tile([128, QW], f32, tag="po", name="po")
                            for cg in range(2):
                                nc.tensor.matmul(
                                    po,
                                    AT8[:, cg, 128 * tb : 128 * (tb + 1)],
                                    wo_sb[:, cg, QW * nn : QW * (nn + 1)],
                                    start=cg == 0,
                                    stop=cg == 1,
                                )
                            ob = p4sb.tile([128, QW], f32, tag="ob", name="ob", bufs=3)
                            if final and cnt % 2 == 1:
                                nc.scalar.copy(ob, po)
                            else:
                                nc.vector.tensor_copy(ob, po)
                            cnt += 1
                            nc.sync.dma_start(
                                out_d[128 * tb : 128 * (tb + 1), QW * nn : QW * (nn + 1)],
                                ob,
                            )

                for j in range(QT):
                    entries = plan[j]
                    if not entries:
                        continue
                    # group entries into aligned DoubleRow pairs (kb=2m, 2m+1)
                    pairs = []
                    i = 0
                    while i < len(entries):
                        e = entries[i]
                        if (
                            i + 1 < len(entries)
                            and e[0] % 2 == 0
                            and entries[i + 1][0] == e[0] + 1
                        ):
                            pairs.append((e, entries[i + 1]))
                            i += 2
                        else:
                            pairs.append((e, None))
                            i += 1
                    qs = slice(QW * j, QW * (j + 1))
                    for g in range(NG):
                        o_ps = psO.tile([128, QW], f32, tag="o", name="o_ps")
                        den_ps = [
                            psA.tile([1, QW], f32, tag=f"den{hl}", name=f"den{hl}", bufs=1)
                            for hl in range(2)
                        ]
                        npair = len(pairs)
                        def emit_av(pt8, ea, eb, qlp, qhp, first, last):
                            if eb is not None:
                                # hl=0 and both denominators: DoubleRow over
                                # the aligned kb pair. hl=1 writes PSUM
                                # partitions 64.. which DoubleRow can't
                                # address (no col tile_position), so it runs
                                # plain-fp8 per entry.
                                m = ea[0] // 2
                                nc.tensor.matmul(
                                    o_ps[0:64, qlp:qhp],
                                    vaug[m][:, :, 2 * g, :],
                                    pt8[:, :, 0, qlp:qhp],
                                    start=first,
                                    stop=last,
                                    perf_mode=DR,
                                    skip_group_check=True,
                                )
                                for hl in range(2):
                                    nc.tensor.matmul(
                                        den_ps[hl][:, qlp:qhp],
                                        ones8[:, :, 0:1],
                                        pt8[:, :, hl, qlp:qhp],
                                        start=first,
                                        stop=last,
                                        perf_mode=DR,
                                        skip_group_check=True,
                                    )
                                for ei2, e2 in enumerate((ea, eb)):
                                    kb2 = e2[0]
                                    nc.tensor.matmul(
                                        o_ps[64:128, qlp:qhp],
                                        vaug[kb2 // 2][:, kb2 % 2, 2 * g + 1, :],
                                        pt8[:, ei2, 1, qlp:qhp],
                                        start=first and ei2 == 0,
                                        stop=last and ei2 == 1,
                                        skip_group_check=True,
                                    )
                            else:
                                kb = ea[0]
                                for hl in range(2):
                                    nc.tensor.matmul(
                                        o_ps[64 * hl : 64 * (hl + 1), qlp:qhp],
                                        vaug[kb // 2][:, kb % 2, 2 * g + hl, :],
                                        pt8[:, 0, hl, qlp:qhp],
                                        start=first,
                                        stop=last,
                                        skip_group_check=True,
                                    )
                                    nc.tensor.matmul(
                                        den_ps[hl][:, qlp:qhp],
                                        ones8[:, kb % 2, 0:1],
                                        pt8[:, 0, hl, qlp:qhp],
                                        start=first,
                                        stop=last,
                                        skip_group_check=True,
                                    )

                        pend_av = None
                        for pi, (ea, eb) in enumerate(pairs):
                            pt8 = p2sb.tile(
                                [128, 2, 2, QW], fp8, tag="pt8", name="pt8", bufs=6
                            )
                            qlp = min(e[1] for e in (ea, eb) if e)
                            qhp = max(e[2] for e in (ea, eb) if e)
                            for ei, e in enumerate((ea, eb)):
                                if e is None:
                                    continue
                                kb, ql, qh, subs = e
                                s_ps = psS.tile(
                                    [128, 2, QW], f32, tag="s", name="s_ps"
                                )
                                # one accumulation group per (entry, hl) bank:
                                # a single start=True S matmul over [ql,qh),
                                # then mask patterns accumulate; the last
                                # instruction carries stop=True (a second
                                # start in the same PSUM bank would re-pend
                                # the whole 2KB zero region on hardware)
                                for hl in range(2):
                                    rs = slice(64 * hl, 64 * (hl + 1))
                                    nc.tensor.matmul(
                                        s_ps[:, hl, ql:qh],
                                        kT[g][rs, 128 * kb : 128 * (kb + 1)],
                                        qT[g][rs, QW * j + ql : QW * j + qh],
                                        start=True,
                                        stop=not subs,
                                        tile_position=(64 * hl, 0),
                                    )
                                    for si, (qb, pidx) in enumerate(subs):
                                        nc.tensor.matmul(
                                            s_ps[:, hl, 128 * qb : 128 * (qb + 1)],
                                            ident,
                                            pats[:, 128 * pidx : 128 * (pidx + 1)],
                                            start=False,
                                            stop=si == len(subs) - 1,
                                        )
                                # zero pt8 outside [ql,qh) within pair range
                                if ql > qlp:
                                    nc.gpsimd.memset(pt8[:, ei, :, qlp:ql], 0.0)
                                if qh < qhp:
                                    nc.gpsimd.memset(pt8[:, ei, :, qh:qhp], 0.0)
                                nc.scalar.activation(
                                    pt8[:, ei, :, ql:qh],
                                    s_ps[:, :, ql:qh],
                                    AF.Exp,
                                    bias=nb_col,
                                    scale=SCALE,
                                )
                            # defer this pair's AV/den behind the next pair's
                            # S matmuls so the in-order PE queue never stalls
                            # on this pair's exp
                            if pend_av is not None:
                                pend_av()
                            pend_av = (
                                lambda pt8=pt8, ea=ea, eb=eb, qlp=qlp, qhp=qhp,
                                first=pi == 0, last=pi == npair - 1:
                                emit_av(pt8, ea, eb, qlp, qhp, first, last)
                            )
                            if pi == 1:
                                emit_oproj()
                        pend_av()
                        # epilogue: AT = o_ps (gated) / den_bcast + vT.
                        # partition_broadcast only writes correctly at dst
                        # base 0, so each head's recip gets a full 128-row
                        # broadcast and the mult runs per 64-row half.
                        tmp16 = p4sb.tile([128, QW], bf16, tag="tmp", name="tmp", bufs=3)
                        for hl in range(2):
                            rd = p4sb.tile([1, QW], f32, tag="rd", name="rd", bufs=4)
                            nc.vector.reciprocal(rd, den_ps[hl])
                            m2sb = p4sb.tile([128, QW], f32, tag="m2s", name="m2s", bufs=2)
                            nc.gpsimd.partition_broadcast(m2sb, rd, channels=128)
                            rs = slice(64 * hl, 64 * (hl + 1))
                            nc.vector.tensor_tensor(
                                tmp16[rs, :], o_ps[rs, :], m2sb[rs, :], OP.mult
                            )
                        nc.gpsimd.tensor_tensor(
                            AT8[:, g, qs], tmp16, vT[g][:, qs], OP.add
                        )
                    pending_oproj.extend(range(4 * j, 4 * j + 4))
                emit_oproj(final=True)

    nc.compile()
    return nc


def kernel(**inputs):
    import ml_dtypes

    from concourse import bass_utils

    NPF8 = ml_dtypes.float8_e4m3fn
    NPBF = ml_dtypes.bfloat16

    x = np.asarray(inputs["x"], np.float32)
    mask = np.asarray(inputs["attention_mask"])
    Wq = np.asarray(inputs["Wq"], np.float32)
    Wk = np.asarray(inputs["Wk"], np.float32)
    Wv = np.asarray(inputs["Wv"], np.float32)
    Wo = np.asarray(inputs["Wo"], np.float32)
    qw = np.asarray(inputs["q_norm_w"], np.float32)
    kw = np.asarray(inputs["k_norm_w"], np.float32)
    gate = np.asarray(inputs["gate"], np.float32).reshape(H)

    mask01 = mask.reshape(T, T) != 0
    plan, patterns = _analyze_mask(mask01)
    npat = patterns.shape[0]

    # shift exp so fp8e4 output can't overflow (cancels in normalization)
    bound = 8.0 * np.max(np.abs(qw)) * np.max(np.abs(kw))
    neg_bias = -max(0.0, float(bound) - 6.0)

    key = (hash(mask01.tobytes()), npat, neg_bias)
    if key not in _CACHE:
        _CACHE[key] = _build_program(plan, npat, neg_bias)
    nc = _CACHE[key]

    pats_r = np.ascontiguousarray(
        patterns.transpose(1, 0, 2).reshape(128, 128 * npat)
    ).astype(NPBF)
    sel2 = np.zeros((128, 2), np.float32)
    sel2[0:64, 0] = 1.0 / D
    sel2[64:128, 1] = 1.0 / D
    sel2 = sel2.astype(NPBF)
    selT = np.zeros((2, 128), np.float32)
    selT[0, 0:64] = 1.0
    selT[1, 64:128] = 1.0
    ident16 = np.eye(128, dtype=np.float32).astype(NPBF)
    wq_col = np.tile(qw, 2)[:, None].astype(np.float32)
    wk_col = np.tile(kw, 2)[:, None].astype(np.float32)

    def chunk_major16(W):
        # [1024, n] -> [128, CB*n] bf16, 128-row chunks side by side
        ci, n = W.shape
        return np.ascontiguousarray(
            W.reshape(CB, 128, n).transpose(1, 0, 2).reshape(128, -1)
        ).astype(NPBF)

    in_maps = []
    for core in range(NCORES):
        b, grp = core // 4, core % 4
        hs = slice(256 * grp, 256 * (grp + 1))
        gcol4 = np.ascontiguousarray(
            np.broadcast_to(gate[4 * grp : 4 * grp + 4][None, :], (128, HPC))
        ).astype(np.float32)
        wo8 = np.ascontiguousarray(
            Wo[hs, :].reshape(2, 128, 1024).transpose(1, 0, 2).reshape(128, -1)
        ).astype(NPBF)
        in_maps.append(
            {
                "x16": chunk_major16(x[b].T),
                "wq16": chunk_major16(Wq[:, hs]),
                "wk16": chunk_major16(Wk[:, hs]),
                "wv16": chunk_major16(Wv[:, hs]),
                "wo8": wo8,
                "wq_col": wq_col,
                "wk_col": wk_col,
                "sel2b": sel2,
                "selTr": selT,
                "gcol4": gcol4,
                "ident16": ident16,
                "patB": pats_r,
            }
        )

    global _LAST_IN_MAPS
    _LAST_IN_MAPS = in_maps
    res = bass_utils.run_bass_kernel_spmd(nc, in_maps, core_ids=list(range(NCORES)))
    parts = [res.results[i]["out"] for i in range(NCORES)]
    out = np.stack(
        [
            parts[0] + parts[1] + parts[2] + parts[3],
            parts[4] + parts[5] + parts[6] + parts[7],
        ]
    )
    return out.astype(np.float32)
